# revision 1
# baseline (speedup 1.0000x reference)
"""ExpertLinear (MoE routing) Trainium2 Bass kernel.

y[b,:] = sum_k ew[b,k] * (x[b,:] @ W[k].T) + (ew @ bias)[b,:]

Strategy: 8-way data-parallel over the batch B across the 8 NeuronCores.
Per core (B_loc = 1024):
  - host supplies layout-prepped shards: xT [IN, B_loc] (x transposed),
    WT [K, IN, OUT] (weights transposed), ewT [K, B_loc], and the
    per-partition routing scalars ewp [128, B_loc/128, K]
  - the blended-expert matmul runs as fp32r accumulations:
        psum[b_tile, :] = sum_i xT[i, b_tile] @ WT[k, i, :]      (per expert k)
        y_acc[b, :]    += ACT(psum * ew[:, k])                   (per-partition scale)
    The second matmul of each (oh0, oh1) PSUM-bank pair reuses the
    stationary operand (ldweights=False) — fp32r matmuls are self-loading
    and the reload costs ~107 ns/MM otherwise.
  - bias term (ewT.T @ bias) is added at the end.
"""

import numpy as np

from concourse import bacc
import concourse.mybir as mybir
import concourse.tile as tile
from concourse.bass_utils import run_bass_kernel_spmd

N_CORES = 8
B, K, OUT, IN = 8192, 8, 1024, 1024
P = 128

# float32r: matmul inputs rounded to 11-bit mantissa, fp32 accumulate;
# streams 1 column/cycle (vs 4 cycles for float32).
MM_DT = mybir.dt.float32r


def build_nc(b_loc=B // N_CORES, k=K, out_dim=OUT, in_dim=IN, mm_dt=MM_DT, rep=1,
             with_bias=True):
    nbt = b_loc // P      # batch tiles per core
    ni = in_dim // P      # contraction subtiles
    oh_sz = 512           # PSUM bank = 512 fp32
    noh = out_dim // oh_sz

    nc = bacc.Bacc()
    xt_d = nc.dram_tensor("xt", [in_dim, b_loc], mm_dt, kind="ExternalInput")
    wt_d = nc.dram_tensor("wt", [k, in_dim, out_dim], mm_dt, kind="ExternalInput")
    ewp_d = nc.dram_tensor("ewp", [P, nbt, k], mybir.dt.float32, kind="ExternalInput")
    ewt_d = nc.dram_tensor("ewt", [k, b_loc], mm_dt, kind="ExternalInput")
    bias_d = nc.dram_tensor("bias", [k, out_dim], mm_dt, kind="ExternalInput")
    y_d = nc.dram_tensor("y", [b_loc, out_dim], mybir.dt.float32, kind="ExternalOutput")

    with tile.TileContext(nc) as tc:
        with (
            tc.tile_pool(name="consts", bufs=1) as consts,
            tc.tile_pool(name="xt", bufs=1) as xt_pool,
            tc.tile_pool(name="yacc", bufs=1) as yacc_pool,
            tc.tile_pool(name="wbuf", bufs=2) as w_pool,
            tc.tile_pool(name="tmp", bufs=4) as tmp_pool,
            tc.tile_pool(name="ps_mm", bufs=4, space="PSUM") as ps_mm_pool,
        ):
            ewp_sb = consts.tile([P, nbt, k], mybir.dt.float32)
            nc.sync.dma_start(ewp_sb[:], ewp_d[:])
            ewt_sb = consts.tile([k, b_loc], mm_dt)
            nc.sync.dma_start(ewt_sb[:], ewt_d[:])
            bias_sb = consts.tile([k, out_dim], mm_dt)
            nc.sync.dma_start(bias_sb[:], bias_d[:])

            # xT resident, one tile per batch-tile so the first matmuls only
            # wait for their own slice: [128 (i_inner), ni (i_outer), P (b)]
            def load_xt(bt):
                xTbt = xt_pool.tile([P, ni, P], mm_dt, name=f"xT{bt}", tag=f"xT{bt}")
                nc.sync.dma_start(
                    xTbt[:],
                    xt_d[:, bt * P:(bt + 1) * P].rearrange("(io p) b -> p io b", p=P),
                )
                return xTbt

            def load_wchunks(kk):
                # W streamed in per-i chunks so matmuls start as soon as the
                # first contraction slice lands
                wchunks = []
                for i in range(ni):
                    wc = w_pool.tile(
                        [P, out_dim], mm_dt, name=f"wc{i}", tag=f"wc{i}"
                    )
                    nc.sync.dma_start(wc[:], wt_d[kk, i * P:(i + 1) * P, :])
                    wchunks.append(wc)
                return wchunks

            # DMA issue order shapes the critical path: xT[0] and expert-0's
            # W chunks go first so the first matmul series starts as early as
            # possible; the remaining batch tiles follow behind.
            xTs = [None] * nbt
            xTs[0] = load_xt(0)
            wchunks_k0 = load_wchunks(0)
            for bt in range(1, nbt):
                xTs[bt] = load_xt(bt)

            y_acc = yacc_pool.tile([P, nbt, out_dim], mybir.dt.float32)

            for _rep in range(rep):
                # Bias seed: y_acc = ewT.T @ bias. These self-loading matmuls
                # are first in program order, so they are all scheduled before
                # any weight-reuse pair below can be split by them. Skipped
                # when the caller knows bias == 0 (expert 0 then writes y_acc
                # directly).
                if with_bias:
                    for bt in range(nbt):
                        pbias = ps_mm_pool.tile(
                            [P, noh, oh_sz], mybir.dt.float32,
                            name="pbias", tag="ps_mm",
                        )
                        for oh in range(noh):
                            nc.tensor.matmul(
                                pbias[:, oh, :],
                                ewt_sb[:, bt * P:(bt + 1) * P],
                                bias_sb[:, oh * oh_sz:(oh + 1) * oh_sz],
                                start=True,
                                stop=True,
                            )
                        for oh in range(noh):
                            nc.scalar.copy(
                                y_acc[:, bt, oh * oh_sz:(oh + 1) * oh_sz],
                                pbias[:, oh, :],
                            )

                # Main loop: stream each expert's WT once; accumulate over
                # the contraction (i) in PSUM, blend over experts (k) into
                # y_acc via ACT per-partition scale + DVE add.
                for kk in range(k):
                    if kk == 0 and _rep == 0:
                        wchunks = wchunks_k0
                    else:
                        wchunks = load_wchunks(kk)
                    for bt in range(nbt):
                        # one PSUM tile spanning both oh banks: the pair's
                        # matmuls share slot state, so the ldweights=False
                        # matmul is always scheduled directly after its
                        # weight-loading partner on the PE queue
                        pss = ps_mm_pool.tile(
                            [P, noh, oh_sz], mybir.dt.float32,
                            name="psmm", tag="ps_mm",
                        )
                        for i in range(ni):
                            lhsT = xTs[bt][:, i, :]
                            for oh in range(noh):
                                nc.tensor.matmul(
                                    pss[:, oh, :],
                                    lhsT,
                                    wchunks[i][:, oh * oh_sz:(oh + 1) * oh_sz],
                                    start=(i == 0),
                                    stop=(i == ni - 1),
                                )
                        for oh in range(noh):
                            osl = y_acc[:, bt, oh * oh_sz:(oh + 1) * oh_sz]
                            scale = ewp_sb[:, bt, kk:kk + 1]
                            if not with_bias and kk == 0:
                                # no bias seed: expert 0 writes y_acc directly
                                nc.scalar.mul(osl, pss[:, oh, :], scale)
                            else:
                                tmp = tmp_pool.tile([P, oh_sz], mybir.dt.float32)
                                nc.scalar.mul(tmp[:], pss[:, oh, :], scale)
                                nc.vector.tensor_add(osl, osl, tmp[:])
                        if kk == k - 1:
                            # y[bt] complete — stream it out while the
                            # remaining batch tiles finish
                            nc.sync.dma_start(
                                y_d[bt * P:(bt + 1) * P, :], y_acc[:, bt, :]
                            )

    nc.compile()

    # Post-compile weight-reuse pass: in the FINAL instruction order, any
    # matmul whose directly-preceding matmul on the PE queue loads the
    # identical stationary AP can skip its fp32r self-load (~107 ns).
    # Done after scheduling/bacc so pairing reflects the real PE order.
    n_reuse = 0
    for blk in nc.m.functions[0].blocks:
        prev_mm = None
        for inst in blk.instructions:
            if isinstance(inst, mybir.InstMatmult):
                if (
                    prev_mm is not None
                    and not inst.is_transpose
                    and not prev_mm.is_transpose
                    and str(prev_mm.ins[1]) == str(inst.ins[1])
                    and prev_mm.tile_position == inst.tile_position
                ):
                    inst.ldweights = False
                    n_reuse += 1
                prev_mm = inst
    return nc


_NC_CACHE = {}


def _get_nc(with_bias=True):
    key = ("default", with_bias)
    if key not in _NC_CACHE:
        _NC_CACHE[key] = build_nc(with_bias=with_bias)
    return _NC_CACHE[key]


def make_in_maps(x, ew, weight, bias):
    b_loc = B // N_CORES
    nbt = b_loc // P
    wt = np.ascontiguousarray(weight.transpose(0, 2, 1))  # [K, IN, OUT]
    in_maps = []
    for c in range(N_CORES):
        xs = x[c * b_loc:(c + 1) * b_loc]
        xt = np.ascontiguousarray(xs.T)  # [IN, b_loc]
        ews = ew[c * b_loc:(c + 1) * b_loc]  # [b_loc, K]
        ewp = np.ascontiguousarray(
            ews.reshape(nbt, P, K).transpose(1, 0, 2)
        )  # [P, nbt, K]
        ewt = np.ascontiguousarray(ews.T)  # [K, b_loc]
        in_maps.append({"xt": xt, "wt": wt, "ewp": ewp, "ewt": ewt, "bias": bias})
    return in_maps


def kernel(x, expert_weights, weight, bias):
    x = np.asarray(x, dtype=np.float32)
    ew = np.asarray(expert_weights, dtype=np.float32)
    weight = np.asarray(weight, dtype=np.float32)
    bias = np.asarray(bias, dtype=np.float32)

    nc = _get_nc(with_bias=bool(np.any(bias)))
    in_maps = make_in_maps(x, ew, weight, bias)
    last_exc = None
    for _attempt in range(3):
        try:
            res = run_bass_kernel_spmd(nc, in_maps, core_ids=list(range(N_CORES)))
            break
        except Exception as exc:  # transient device errors: retry
            last_exc = exc
    else:
        raise last_exc
    y = np.concatenate([r["y"] for r in res.results], axis=0)
    return y

